# revision 1
# baseline (speedup 1.0000x reference)
"""CSWin-style cross-attention block for Trainium2 (Bass/Tile), 8-core data-parallel.

Per core (one batch image, L=4096=64x64, C=256):
  qkv = x @ qkv_w; 4 branch attentions on half-channels with strip windows
  (64x8 / 8x64), depthwise-conv LePE added to attention out; concat; proj.

Key mapping decisions:
  - feature-major q/k/v [C, L] (bf16) via PE-transpose of x + feature-major QKV.
  - scores computed transposed: S^T[kpix, q] per (branch, window, head,
    kchunk), 4 heads row-packed (K=32 at array rows 32h).
  - exp on ScalarE (softmax scale folded into activation pre-scale), bf16 out.
  - AV col-packed 4 heads (M=32 at cols 32h); V^T via PE transposes.
  - denominators via ones-vector M=1 matmuls; recip via DVE
    reciprocal_approx_fast on the raw PSUM bank; GpSimd partition_broadcast.
  - LePE 3x3 depthwise conv: 9 diagonal-weight matmuls over a zero-padded
    flat window layout (every tap is a 1-D shifted span; PE matmul operands
    allow only one free dim).
  - proj per 128-token chunk: 4 branch matmuls accumulated in PSUM + bias.
"""
import os
import sys

sys.path.insert(0, "/opt/trn_rl_repo")
import numpy as np
import ml_dtypes

import concourse.bacc as bacc
import concourse.mybir as mybir
import concourse.tile as tile
from concourse.bass_utils import run_bass_kernel_spmd
from concourse.masks import make_identity

BF = mybir.dt.bfloat16
F32 = mybir.dt.float32
AF = mybir.ActivationFunctionType
ALU = mybir.AluOpType
SCALE = float(32.0 ** -0.5)

# tap order: (0,0) first so the start=True matmul covers the whole region
TAPS = [(0, 0)] + [(dr, dj) for dr in (-1, 0, 1) for dj in (-1, 0, 1) if (dr, dj) != (0, 0)]

# branch -> (combo, qhalf, kvhalf); combo A = 64x8 windows, B = 8x64
BRANCH = {0: ("A", 0, 0), 1: ("B", 1, 1), 2: ("A", 1, 0), 3: ("B", 0, 1)}

# padded flat window layouts for LePE: (rows, cols, row_pitch, region_base, total)
PAD = {"A": (64, 8, 10, 16, 672), "B": (8, 64, 66, 80, 688)}


def build(nc, debug=False, repeat=1, dyn_loop=0, with_cbias=True):
    xb = nc.dram_tensor("xb", [4096, 256], BF, kind="ExternalInput").ap()
    qw = nc.dram_tensor("qw", [256, 768], BF, kind="ExternalInput").ap()
    pw = nc.dram_tensor("pw", [512, 256], BF, kind="ExternalInput").ap()
    dg = nc.dram_tensor("dg", [36, 128, 128], BF, kind="ExternalInput").ap()
    pb = nc.dram_tensor("pb", [128, 256], F32, kind="ExternalInput").ap()
    cb = nc.dram_tensor("cb", [128, 4], F32, kind="ExternalInput").ap()
    sel = nc.dram_tensor("sel", [128, 128], F32, kind="ExternalInput").ap()
    out_d = nc.dram_tensor("out", [4096, 256], F32, kind="ExternalOutput").ap()
    dbg = {}
    if debug:
        for name, shape in [("d_sc", [128, 2048]), ("d_exp", [128, 2048]),
                            ("d_av", [128, 512]), ("d_den", [128, 512]),
                            ("d_lep", [128, 688]), ("d_cat", [128, 512]),
                            ("d_vt", [128, 512])]:
            dbg[name] = nc.dram_tensor(name, shape, F32, kind="ExternalOutput").ap()

    with tile.TileContext(nc) as tc:
        with tc.sbuf_pool(name="persist", bufs=1) as ps_pool:
            # ---- constants / weights ----
            ident = ps_pool.tile([128, 128], BF, name="ident")
            make_identity(nc, ident)
            ones_f = ps_pool.tile([128, 1], F32, name="ones_f")
            nc.vector.memset(ones_f, 1.0)
            ones_b = ps_pool.tile([128, 1], BF, name="ones_b")
            nc.vector.tensor_copy(ones_b, ones_f)
            ones512 = ps_pool.tile([128, 512], BF, name="ones512")
            nc.vector.memset(ones512, 1.0)
            cbdiag = ps_pool.tile([128, 4 * 128], BF, name="cbdiag")

            qw_t = [ps_pool.tile([128, 768], BF, name=f"qw{i}") for i in range(2)]
            for i in range(2):
                nc.sync.dma_start(qw_t[i], qw[128 * i:128 * (i + 1), :])
            pw_t = [ps_pool.tile([128, 256], BF, name=f"pw{i}") for i in range(4)]
            for i in range(4):
                nc.sync.dma_start(pw_t[i], pw[128 * i:128 * (i + 1), :])
            diag_t = ps_pool.tile([128, 36 * 128], BF, name="diag_t")
            nc.sync.dma_start(diag_t.rearrange("p (t c) -> p t c", t=36),
                              dg.rearrange("t p c -> p t c"))
            pb_t = ps_pool.tile([128, 256], F32, name="pb_t")
            nc.sync.dma_start(pb_t, pb)
            cb_t = ps_pool.tile([128, 4], F32, name="cb_t")
            nc.sync.dma_start(cb_t, cb)
            sel_t = ps_pool.tile([128, 128], F32, name="sel_t")
            nc.sync.dma_start(sel_t, sel)
            for _b in range(4):
                nc.vector.tensor_scalar(cbdiag[:, 128 * _b:128 * (_b + 1)], ident,
                                        cb_t[:, _b:_b + 1], None, ALU.mult)

            # ---- persistent activations ----
            q_t = [ps_pool.tile([128, 4096], BF, name=f"q{i}") for i in range(2)]
            k_t = [ps_pool.tile([128, 4096], BF, name=f"k{i}") for i in range(2)]
            v_t = [ps_pool.tile([128, 4096], BF, name=f"v{i}") for i in range(2)]
            cat_t = [ps_pool.tile([128, 4096], BF, name=f"cat{i}") for i in range(4)]

            def _emit(_rep):
                # ================= phase 0: x^T + QKV =================
                with tc.sbuf_pool(name=f"p0sb{_rep}", bufs=1) as p0sb, \
                     tc.tile_pool(name=f"p0ps{_rep}", bufs=4, space="PSUM") as p0ps, \
                     tc.tile_pool(name=f"p0ps2{_rep}", bufs=4, space="PSUM") as p0ps2, \
                     tc.sbuf_pool(name=f"p0in{_rep}", bufs=4) as p0in:
                    xT = [p0sb.tile([128, 4096], BF, name=f"xT{i}") for i in range(2)]
                    for n in range(32):
                        xin = p0in.tile([128, 256], BF, tag="xin", name="xin")
                        nc.sync.dma_start(xin, xb[128 * n:128 * (n + 1), :])
                        for cc in range(2):
                            tp = p0ps.tile([128, 128], BF, tag="tp", name="tp")
                            nc.tensor.transpose(tp, xin[:, 128 * cc:128 * (cc + 1)], ident)
                            nc.vector.tensor_copy(xT[cc][:, 128 * n:128 * (n + 1)], tp)
                    for n in range(8):
                        for m in range(6):
                            qp = p0ps2.tile([128, 512], F32, tag="qp", name="qp")
                            for cc in range(2):
                                nc.tensor.matmul(qp, qw_t[cc][:, 128 * m:128 * (m + 1)],
                                                 xT[cc][:, 512 * n:512 * (n + 1)],
                                                 start=(cc == 0), stop=(cc == 1),
                                                 skip_group_check=True)
                            dst = [q_t, k_t, v_t][m // 2][m % 2]
                            # ScalarE copy: DVE is a co-bottleneck, ACT has slack
                            nc.scalar.copy(dst[:, 512 * n:512 * (n + 1)], qp)

                # window views (for DVE staging copies only; matmuls need 1-D free)
                def winview(t, combo):
                    if combo == "A":
                        return t.rearrange("c (r w j) -> c w r j", r=64, w=8, j=8)
                    return t.rearrange("c (w i cc) -> c w i cc", w=8, i=8, cc=64)

                # ================= attention phases =================
                with tc.tile_pool(name=f"scps{_rep}", bufs=2, space="PSUM") as scps, \
                     tc.tile_pool(name=f"avps{_rep}", bufs=1, space="PSUM") as avps, \
                     tc.tile_pool(name=f"auxps{_rep}", bufs=1, space="PSUM") as auxps, \
                     tc.sbuf_pool(name=f"expsb{_rep}", bufs=6) as expsb, \
                     tc.sbuf_pool(name=f"stg{_rep}", bufs=2) as stg, \
                     tc.sbuf_pool(name=f"outsb{_rep}", bufs=4) as outsb:

                    for combo, branches in [("B", (1, 3)), ("A", (0, 2))]:
                        R, J, T, RB, TOT = PAD[combo]
                        kvhalf = BRANCH[branches[0]][2]
                        for w in range(8):
                            is_dbg_w = debug and combo == "A" and w == 0

                            # ---- stage contiguous window copies (combo A only) ----
                            if combo == "A":
                                kwin = stg.tile([128, 512], BF, tag="kwin", name="kwin")
                                nc.vector.tensor_copy(
                                    kwin.rearrange("c (r j) -> c r j", j=8),
                                    winview(k_t[kvhalf], "A")[:, w])
                                vwin = stg.tile([128, 512], BF, tag="vwin", name="vwin")
                                nc.vector.tensor_copy(
                                    vwin.rearrange("c (r j) -> c r j", j=8),
                                    winview(v_t[kvhalf], "A")[:, w])
                                qwin = {}
                                for qh in set(BRANCH[b][1] for b in branches):
                                    qt = stg.tile([128, 512], BF, tag=f"qwin{qh}", name="qwin")
                                    nc.vector.tensor_copy(
                                        qt.rearrange("c (r j) -> c r j", j=8),
                                        winview(q_t[qh], "A")[:, w])
                                    qwin[qh] = qt
                            else:
                                kwin = k_t[kvhalf][:, 512 * w:512 * (w + 1)]
                                vwin = v_t[kvhalf][:, 512 * w:512 * (w + 1)]
                                qwin = {qh: q_t[qh][:, 512 * w:512 * (w + 1)]
                                        for qh in set(BRANCH[b][1] for b in branches)}

                            # ---- zero-padded v window for LePE ----
                            vpad = stg.tile([128, TOT], BF, tag="vpad", name="vpad")
                            nc.vector.memset(vpad, 0.0)
                            nc.vector.tensor_copy(
                                vpad[:, RB:RB + R * T].rearrange(
                                    "c (r t) -> c r t", t=T)[:, :, 1:1 + J],
                                vwin.rearrange("c (r j) -> c r j", j=J))

                            # ---- V^T tiles (shared by the branch pair) ----
                            # layout per kchunk: 4 groups of 64 cols: [vt_head(32) | ones(32)]
                            vt_sb = stg.tile([128, 1024], BF, tag="vt", name="vt_sb")
                            nc.vector.memset(vt_sb, 1.0)
                            vtp = auxps.tile([128, 512], BF, tag="aux", name="vtp")
                            for kc in range(4):
                                nc.tensor.transpose(vtp[:, 128 * kc:128 * (kc + 1)],
                                                    vwin[:, 128 * kc:128 * (kc + 1)], ident)
                            for kc in range(4):
                                dstv = vt_sb[:, 256 * kc:256 * (kc + 1)].rearrange(
                                    "p (g c) -> p g c", g=4)[:, :, 0:32]
                                srcv = vtp[:, 128 * kc:128 * (kc + 1)].rearrange(
                                    "p (g c) -> p g c", g=4)
                                nc.vector.tensor_copy(dstv, srcv)
                            if is_dbg_w:
                                vt32 = stg.tile([128, 512], F32, tag="dbgvt", name="vt32")
                                nc.vector.tensor_copy(vt32, vt_sb)
                                nc.sync.dma_start(dbg["d_vt"], vt32)

                            for br in branches:
                                _, qhalf, kvh = BRANCH[br]
                                qfull = qwin[qhalf]
                                is_dbg = debug and br == 0 and w == 0

                                # --- scores + exp ---
                                exp_tiles = {}
                                for kc in range(4):
                                    sct = [scps.tile([128, 1024], F32, tag="sc", name="sct")
                                           for _ in range(2)]
                                    for h in range(4):
                                        nc.tensor.matmul(
                                            sct[h // 2][:, 512 * (h % 2):512 * (h % 2) + 512],
                                            kwin[32 * h:32 * (h + 1), 128 * kc:128 * (kc + 1)],
                                            qfull[32 * h:32 * (h + 1), :],
                                            start=True, stop=True,
                                            tile_position=(32 * h, 0))
                                    for p in range(2):
                                        e = expsb.tile([128, 1024], BF, tag="exp", name="exp")
                                        nc.scalar.activation(e, sct[p], AF.Exp, scale=SCALE)
                                        exp_tiles[(p, kc)] = e
                                    if is_dbg and kc == 0:
                                        d = stg.tile([128, 2048], F32, tag="dbgsc", name="dsc")
                                        nc.vector.tensor_copy(d[:, :1024], sct[0])
                                        nc.vector.tensor_copy(d[:, 1024:], sct[1])
                                        nc.sync.dma_start(dbg["d_sc"], d)

                                # --- AV + den fused: lhsT = [vt_head | ones] (M=64) ---
                                # out rows 0..32 = attention out, 32..64 = den replicated
                                avden = avps.tile([128, 1024], F32, tag="av", name="avden")
                                for kc in range(4):
                                    for g in range(4):
                                        nc.tensor.matmul(
                                            avden[64 * (g % 2):64 * (g % 2) + 64,
                                                  512 * (g // 2):512 * (g // 2) + 512],
                                            vt_sb[:, 256 * kc + 64 * g:256 * kc + 64 * (g + 1)],
                                            exp_tiles[(g // 2, kc)][:, 512 * (g % 2):512 * (g % 2) + 512],
                                            start=(kc == 0), stop=(kc == 3),
                                            tile_position=(0, 64 * (g % 2)), skip_group_check=True)

                                # --- LePE: 9 diagonal matmuls over padded flat layout ---
                                # (split at PSUM bank boundaries: matmul out must fit 1 bank)
                                lp = auxps.tile([128, TOT], F32, tag="aux", name="lp")
                                span = R * T
                                segs = []
                                s0 = RB
                                while s0 < RB + span:
                                    s1 = min((s0 // 512 + 1) * 512, RB + span)
                                    segs.append((s0, s1))
                                    s0 = s1
                                for t, (dr, dj) in enumerate(TAPS):
                                    delta = T * dr + dj
                                    dmat = diag_t[:, (br * 9 + t) * 128:(br * 9 + t + 1) * 128]
                                    for (s0, s1) in segs:
                                        nc.tensor.matmul(
                                            lp[:, s0:s1],
                                            dmat,
                                            vpad[:, s0 + delta:s1 + delta],
                                            start=(t == 0),
                                            stop=(not with_cbias and t == 8),
                                            skip_group_check=True)
                                if with_cbias:
                                    for (s0, s1) in segs:  # conv bias
                                        nc.tensor.matmul(
                                            lp[:, s0:s1],
                                            cbdiag[:, 128 * br:128 * (br + 1)],
                                            ones512[:, 0:s1 - s0],
                                            start=False, stop=True, skip_group_check=True)

                                # --- normalize (shifted ops) + lepe -> concat ---
                                rd = stg.tile([128, 1024], F32, tag="recip", name="rd")
                                nc.vector.reciprocal_approx_fast(rd, avden)
                                t_sb = stg.tile([128, 1024], F32, tag="tsb", name="t_sb")
                                for half in range(2):
                                    for gg in range(2):
                                        nc.vector.tensor_mul(
                                            t_sb[64 * gg:64 * gg + 32, 512 * half:512 * (half + 1)],
                                            avden[64 * gg:64 * gg + 32, 512 * half:512 * (half + 1)],
                                            rd[64 * gg + 32:64 * gg + 64, 512 * half:512 * (half + 1)])
                                catw = winview(cat_t[br], combo)[:, w]       # [c, R, J]
                                span = R * T
                                lpv = lp[:, RB:RB + span].rearrange(
                                    "c (r t) -> c r t", t=T)[:, :, 1:1 + J]  # [c, R, J]
                                t3 = t_sb.rearrange("c (m a b) -> c m a b", m=2, a=R, b=J)
                                for g in range(4):
                                    nc.vector.tensor_add(
                                        catw[32 * g:32 * (g + 1)],
                                        lpv[32 * g:32 * (g + 1)],
                                        t3[64 * (g % 2):64 * (g % 2) + 32, g // 2])

                                if is_dbg:
                                    for nm, src_t, ln in [("d_av", avden[:, 0:512], 512),
                                                          ("d_den", avden[:, 512:1024], 512),
                                                          ("d_lep", lp, TOT)]:
                                        d = stg.tile([128, 688], F32, tag="dbg" + nm, name="dT")
                                        nc.vector.tensor_copy(d[:, :ln], src_t)
                                        nc.sync.dma_start(dbg[nm][:, :ln], d[:, :ln])
                                    d1 = stg.tile([128, 2048], F32, tag="dbgexp", name="dexp")
                                    nc.vector.tensor_copy(d1[:, :1024], exp_tiles[(0, 0)])
                                    nc.vector.tensor_copy(d1[:, 1024:], exp_tiles[(1, 0)])
                                    nc.sync.dma_start(dbg["d_exp"], d1)
                                    dc = stg.tile([128, 512], F32, tag="dbgcat", name="dcat")
                                    nc.vector.tensor_copy(
                                        dc.rearrange("c (a b) -> c a b", a=R, b=J), catw)
                                    nc.sync.dma_start(dbg["d_cat"], dc)

                        # (proj moved to tail: combo A runs last and spans all tokens)
                    for n in range(32):
                        pp = auxps.tile([128, 256], F32, tag="aux", name="pp")
                        for b2 in range(4):
                            nc.tensor.matmul(pp, cat_t[b2][:, 128 * n:128 * (n + 1)],
                                             pw_t[b2], start=(b2 == 0), stop=(b2 == 3),
                                             skip_group_check=True)
                        osb = outsb.tile([128, 256], F32, tag="out", name="osb")
                        nc.vector.tensor_add(osb, pp, pb_t)
                        nc.sync.dma_start(out_d[128 * n:128 * (n + 1), :], osb)

            if dyn_loop:
                with tc.For_i(0, dyn_loop, 1):
                    _emit(0)
            else:
                for _rep in range(repeat):
                    _emit(_rep)

    return nc


_CACHE = {}


def _get_nc(debug=False, repeat=1, dyn_loop=0, with_cbias=True):
    key = (bool(debug), repeat, dyn_loop, with_cbias)
    if key not in _CACHE:
        nc = bacc.Bacc("TRN2", target_bir_lowering=False, debug=False)
        build(nc, debug=debug, repeat=repeat, dyn_loop=dyn_loop, with_cbias=with_cbias)
        nc.compile()
        _CACHE[key] = nc
    return _CACHE[key]


def prep_inputs(x, qkv_w, proj_w, proj_b, conv_ws, conv_bs):
    x = np.asarray(x)
    B = x.shape[0]
    xb = x.astype(ml_dtypes.bfloat16)
    qwb = np.asarray(qkv_w).astype(ml_dtypes.bfloat16)
    pwb = np.asarray(proj_w).astype(ml_dtypes.bfloat16)
    w9 = np.asarray(conv_ws).reshape(4, 128, 9).astype(np.float32)
    dgn = np.zeros((36, 128, 128), np.float32)
    idx = np.arange(128)
    for br in range(4):
        for t, (dr, dj) in enumerate(TAPS):
            dgn[br * 9 + t, idx, idx] = w9[br, :, (dr + 1) * 3 + (dj + 1)]
    dgn = dgn.astype(ml_dtypes.bfloat16)
    pbb = np.tile(np.asarray(proj_b, np.float32)[None, :], (128, 1))
    cbt = np.ascontiguousarray(np.asarray(conv_bs, np.float32).T)
    seln = np.zeros((128, 128), np.float32)
    for c in range(128):
        seln[32 * (c // 32), c] = 1.0
    shared = {"qw": qwb, "pw": pwb, "dg": dgn, "pb": pbb, "cb": cbt, "sel": seln}
    return [dict(shared, xb=np.ascontiguousarray(xb[b])) for b in range(B)]


def kernel(x, qkv_w, proj_w, proj_b, conv_ws, conv_bs, _debug=False, _trace=False):
    wcb = bool(np.any(np.asarray(conv_bs)))
    nc = _get_nc(debug=_debug, with_cbias=wcb)
    in_maps = prep_inputs(x, qkv_w, proj_w, proj_b, conv_ws, conv_bs)
    res = run_bass_kernel_spmd(nc, in_maps, core_ids=list(range(len(in_maps))),
                               trace=_trace)
    out = np.stack([r["out"] for r in res.results]).astype(np.float32)
    if _debug or _trace:
        kernel.last_results = res
    return out



# revision 6
# speedup vs baseline: 1.1933x; 1.1933x over previous
"""CSWin-style cross-attention block for Trainium2 (Bass/Tile), 8-core data-parallel.

Per core (one batch image, L=4096=64x64, C=256):
  qkv = x @ qkv_w; 4 branch attentions on half-channels with strip windows
  (64x8 / 8x64), depthwise-conv LePE added to attention out; concat; proj.

v2 mapping (ACT-exp is the bottleneck engine; everything else hides under it):
  - feature-major q/k/v [C, L] (bf16) via PE-transpose of x + feature-major QKV.
  - scores transposed: S^T[kpix, q] per (branch, window, head, kchunk),
    4 heads row-packed (K=32 at array rows 32h).
  - exp on ScalarE only (scale folded into activation pre-scale), bf16 out.
  - AV 4-way col-tiled: per (head, kc) separate att (lhsT=V^T slice, M=32) and
    den (lhsT=ones, M=32) matmuls at array cols 32*(h%2) / 64+32*(h%2).
    avden rows = [att_h | att_h' | den_h | den_h'] per 512-col head-pair half,
    so normalize is ONE recip [64,1024] + 2 muls/adds on [64,512] slices.
  - LePE 3x3 depthwise conv: 9 diagonal-weight matmuls over a zero-padded
    flat window layout with minimal pitch (T=J+1: one shared pad col between
    rows serves both dj=+-1). Pad borders zeroed once (persistent tiles).
  - proj per 128-token chunk: 4 branch matmuls accumulated in PSUM + bias.
"""
import os
import sys

sys.path.insert(0, "/opt/trn_rl_repo")
import numpy as np
import ml_dtypes

import concourse.bacc as bacc
import concourse.mybir as mybir
import concourse.tile as tile
from concourse.bass_utils import run_bass_kernel_spmd
from concourse.masks import make_identity

BF = mybir.dt.bfloat16
F32 = mybir.dt.float32
AF = mybir.ActivationFunctionType
ALU = mybir.AluOpType
SCALE = float(32.0 ** -0.5)

# tap order: (0,0) first so the start=True matmul covers the whole region
TAPS = [(0, 0)] + [(dr, dj) for dr in (-1, 0, 1) for dj in (-1, 0, 1) if (dr, dj) != (0, 0)]

# branch -> (combo, qhalf, kvhalf); combo A = 64x8 windows, B = 8x64
BRANCH = {0: ("A", 0, 0), 1: ("B", 1, 1), 2: ("A", 1, 0), 3: ("B", 0, 1)}

# padded flat window layouts for LePE: (rows, cols, row_pitch, region_base, total)
# pitch = cols+1: single shared pad col between consecutive rows covers dj=+-1.
PAD = {"A": (64, 8, 9, 16, 608), "B": (8, 64, 65, 68, 656)}


def build(nc, debug=False, repeat=1, dyn_loop=0, with_cbias=True):
    xb = nc.dram_tensor("xb", [4096, 256], BF, kind="ExternalInput").ap()
    qw = nc.dram_tensor("qw", [256, 768], BF, kind="ExternalInput").ap()
    pw = nc.dram_tensor("pw", [512, 256], BF, kind="ExternalInput").ap()
    dg = nc.dram_tensor("dg", [36, 128, 128], BF, kind="ExternalInput").ap()
    pb = nc.dram_tensor("pb", [128, 256], F32, kind="ExternalInput").ap()
    cb = nc.dram_tensor("cb", [128, 4], F32, kind="ExternalInput").ap()
    out_d = nc.dram_tensor("out", [4096, 256], F32, kind="ExternalOutput").ap()
    dbg = {}
    if debug:
        for name, shape in [("d_sc", [128, 2048]), ("d_exp", [128, 2048]),
                            ("d_av", [128, 1024]), ("d_rd", [128, 1024]),
                            ("d_lep", [128, 656]), ("d_cat", [128, 512]),
                            ("d_vt", [128, 512])]:
            dbg[name] = nc.dram_tensor(name, shape, F32, kind="ExternalOutput").ap()

    with tile.TileContext(nc) as tc:
        with tc.sbuf_pool(name="persist", bufs=1) as ps_pool:
            # ---- constants / weights ----
            ident = ps_pool.tile([128, 128], BF, name="ident")
            make_identity(nc, ident)
            ones32 = ps_pool.tile([128, 32], BF, name="ones32")
            nc.vector.memset(ones32, 1.0)
            ones512 = ps_pool.tile([128, 512], BF, name="ones512")
            nc.vector.memset(ones512, 1.0)
            cbdiag = ps_pool.tile([128, 4 * 128], BF, name="cbdiag")

            qw_t = [ps_pool.tile([128, 768], BF, name=f"qw{i}") for i in range(2)]
            for i in range(2):
                nc.sync.dma_start(qw_t[i], qw[128 * i:128 * (i + 1), :])
            pw_t = [ps_pool.tile([128, 256], BF, name=f"pw{i}") for i in range(4)]
            for i in range(4):
                nc.sync.dma_start(pw_t[i], pw[128 * i:128 * (i + 1), :])
            diag_t = ps_pool.tile([128, 36 * 128], BF, name="diag_t")
            nc.sync.dma_start(diag_t.rearrange("p (t c) -> p t c", t=36),
                              dg.rearrange("t p c -> p t c"))
            pb_t = ps_pool.tile([128, 256], F32, name="pb_t")
            nc.sync.dma_start(pb_t, pb)
            cb_t = ps_pool.tile([128, 4], F32, name="cb_t")
            nc.sync.dma_start(cb_t, cb)
            for _b in range(4):
                nc.vector.tensor_scalar(cbdiag[:, 128 * _b:128 * (_b + 1)], ident,
                                        cb_t[:, _b:_b + 1], None, ALU.mult)

            # ---- persistent activations ----
            q_t = [ps_pool.tile([128, 4096], BF, name=f"q{i}") for i in range(2)]
            k_t = [ps_pool.tile([128, 4096], BF, name=f"k{i}") for i in range(2)]
            v_t = [ps_pool.tile([128, 4096], BF, name=f"v{i}") for i in range(2)]
            cat_t = [ps_pool.tile([128, 4096], BF, name=f"cat{i}") for i in range(4)]

            # persistent zero-padded LePE staging (borders zeroed once,
            # interiors rewritten per window; 2 bufs each for overlap)
            vpads = {}
            for combo in ("A", "B"):
                R, J, T, RB, TOT = PAD[combo]
                tiles = [ps_pool.tile([128, TOT], BF, name=f"vpad{combo}{i}")
                         for i in range(2)]
                for t in tiles:
                    nc.vector.memset(t, 0.0)
                vpads[combo] = tiles

            def _emit(_rep):
                # ================= phase 0: x^T + QKV =================
                with tc.sbuf_pool(name=f"p0sb{_rep}", bufs=1) as p0sb, \
                     tc.tile_pool(name=f"p0ps{_rep}", bufs=4, space="PSUM") as p0ps, \
                     tc.tile_pool(name=f"p0ps2{_rep}", bufs=4, space="PSUM") as p0ps2, \
                     tc.sbuf_pool(name=f"p0in{_rep}", bufs=4) as p0in:
                    xT = [p0sb.tile([128, 4096], BF, name=f"xT{i}") for i in range(2)]
                    for n in range(32):
                        xin = p0in.tile([128, 256], BF, tag="xin", name="xin")
                        nc.sync.dma_start(xin, xb[128 * n:128 * (n + 1), :])
                        for cc in range(2):
                            tp = p0ps.tile([128, 128], BF, tag="tp", name="tp")
                            nc.tensor.transpose(tp, xin[:, 128 * cc:128 * (cc + 1)], ident)
                            nc.vector.tensor_copy(xT[cc][:, 128 * n:128 * (n + 1)], tp)
                    for n in range(8):
                        for m in range(6):
                            qp = p0ps2.tile([128, 512], F32, tag="qp", name="qp")
                            for cc in range(2):
                                nc.tensor.matmul(qp, qw_t[cc][:, 128 * m:128 * (m + 1)],
                                                 xT[cc][:, 512 * n:512 * (n + 1)],
                                                 start=(cc == 0), stop=(cc == 1),
                                                 skip_group_check=True)
                            dst = [q_t, k_t, v_t][m // 2][m % 2]
                            # ScalarE copy: ACT idle in phase 0, DVE busy with xT
                            nc.scalar.copy(dst[:, 512 * n:512 * (n + 1)], qp)

                # window views (for DVE staging copies only; matmuls need 1-D free)
                def winview(t, combo):
                    if combo == "A":
                        return t.rearrange("c (r w j) -> c w r j", r=64, w=8, j=8)
                    return t.rearrange("c (w i cc) -> c w i cc", w=8, i=8, cc=64)

                # ================= attention phases =================
                with tc.tile_pool(name=f"scps{_rep}", bufs=2, space="PSUM") as scps, \
                     tc.tile_pool(name=f"avps{_rep}", bufs=1, space="PSUM") as avps, \
                     tc.tile_pool(name=f"auxps{_rep}", bufs=1, space="PSUM") as auxps, \
                     tc.sbuf_pool(name=f"expsb{_rep}", bufs=6) as expsb, \
                     tc.sbuf_pool(name=f"stg{_rep}", bufs=2) as stg, \
                     tc.sbuf_pool(name=f"outsb{_rep}", bufs=4) as outsb:

                    for combo, branches in [("B", (1, 3)), ("A", (0, 2))]:
                        R, J, T, RB, TOT = PAD[combo]
                        kvhalf = BRANCH[branches[0]][2]
                        for w in range(8):
                            # ---- stage contiguous window copies (combo A only) ----
                            if combo == "A":
                                kwin = stg.tile([128, 512], BF, tag="kwin", name="kwin")
                                nc.vector.tensor_copy(
                                    kwin.rearrange("c (r j) -> c r j", j=8),
                                    winview(k_t[kvhalf], "A")[:, w])
                                vwin = stg.tile([128, 512], BF, tag="vwin", name="vwin")
                                nc.vector.tensor_copy(
                                    vwin.rearrange("c (r j) -> c r j", j=8),
                                    winview(v_t[kvhalf], "A")[:, w])
                                qwin = {}
                                for qh in set(BRANCH[b][1] for b in branches):
                                    qt = stg.tile([128, 512], BF, tag=f"qwin{qh}", name="qwin")
                                    nc.vector.tensor_copy(
                                        qt.rearrange("c (r j) -> c r j", j=8),
                                        winview(q_t[qh], "A")[:, w])
                                    qwin[qh] = qt
                            else:
                                kwin = k_t[kvhalf][:, 512 * w:512 * (w + 1)]
                                vwin = v_t[kvhalf][:, 512 * w:512 * (w + 1)]
                                qwin = {qh: q_t[qh][:, 512 * w:512 * (w + 1)]
                                        for qh in set(BRANCH[b][1] for b in branches)}

                            # ---- zero-padded v window for LePE (interior only) ----
                            vpad = vpads[combo][w % 2]
                            nc.vector.tensor_copy(
                                vpad[:, RB:RB + R * T].rearrange(
                                    "c (r t) -> c r t", t=T)[:, :, 0:J],
                                vwin.rearrange("c (r j) -> c r j", j=J))

                            # ---- V^T: 4 PE transposes + one contiguous copy ----
                            # vt_sb cols = [kc0: ch0..127 | kc1: ... | kc2 | kc3]
                            vtp = auxps.tile([128, 512], BF, tag="aux", name="vtp")
                            for kc in range(4):
                                nc.tensor.transpose(vtp[:, 128 * kc:128 * (kc + 1)],
                                                    vwin[:, 128 * kc:128 * (kc + 1)], ident)
                            vt_sb = stg.tile([128, 512], BF, tag="vt", name="vt_sb")
                            nc.vector.tensor_copy(vt_sb, vtp)
                            if debug and combo == "A" and w == 0:
                                vt32 = stg.tile([128, 512], F32, tag="dbgvt", name="vt32")
                                nc.vector.tensor_copy(vt32, vt_sb)
                                nc.sync.dma_start(dbg["d_vt"], vt32)

                            for br in branches:
                                _, qhalf, kvh = BRANCH[br]
                                qfull = qwin[qhalf]
                                is_dbg = debug and br == 0 and w == 0

                                # --- scores + exp ---
                                exp_tiles = {}
                                for kc in range(4):
                                    sct = [scps.tile([128, 1024], F32, tag="sc", name="sct")
                                           for _ in range(2)]
                                    for h in range(4):
                                        nc.tensor.matmul(
                                            sct[h // 2][:, 512 * (h % 2):512 * (h % 2) + 512],
                                            kwin[32 * h:32 * (h + 1), 128 * kc:128 * (kc + 1)],
                                            qfull[32 * h:32 * (h + 1), :],
                                            start=True, stop=True,
                                            tile_position=(32 * h, 0))
                                    for p in range(2):
                                        e = expsb.tile([128, 1024], BF, tag="exp", name="exp")
                                        nc.scalar.activation(e, sct[p], AF.Exp, scale=SCALE)
                                        exp_tiles[(p, kc)] = e
                                    if is_dbg and kc == 0:
                                        d = stg.tile([128, 2048], F32, tag="dbgsc", name="dsc")
                                        nc.vector.tensor_copy(d[:, :1024], sct[0])
                                        nc.vector.tensor_copy(d[:, 1024:], sct[1])
                                        nc.sync.dma_start(dbg["d_sc"], d)

                                # --- AV + den, 4-way col-tiled ---
                                # avden rows per 512-col half hp (heads 2hp, 2hp+1):
                                #   [0:32]  att head 2hp      [32:64] att head 2hp+1
                                #   [64:96] den head 2hp      [96:128] den head 2hp+1
                                avden = avps.tile([128, 1024], F32, tag="av", name="avden")
                                for kc in range(4):
                                    for h in range(4):
                                        hp, hs = h // 2, h % 2
                                        ecols = exp_tiles[(hp, kc)][:, 512 * hs:512 * hs + 512]
                                        nc.tensor.matmul(
                                            avden[32 * hs:32 * hs + 32,
                                                  512 * hp:512 * hp + 512],
                                            vt_sb[:, 128 * kc + 32 * h:128 * kc + 32 * h + 32],
                                            ecols,
                                            start=(kc == 0), stop=(kc == 3),
                                            tile_position=(0, 32 * hs),
                                            skip_group_check=True)
                                        nc.tensor.matmul(
                                            avden[64 + 32 * hs:64 + 32 * hs + 32,
                                                  512 * hp:512 * hp + 512],
                                            ones32,
                                            ecols,
                                            start=(kc == 0), stop=(kc == 3),
                                            tile_position=(0, 64 + 32 * hs),
                                            skip_group_check=True)

                                # --- LePE: 9 diagonal matmuls over padded flat layout ---
                                # (split at PSUM bank boundaries: matmul out must fit 1 bank)
                                lp = auxps.tile([128, TOT], F32, tag="aux", name="lp")
                                span = R * T
                                segs = []
                                s0 = RB
                                while s0 < RB + span:
                                    s1 = min((s0 // 512 + 1) * 512, RB + span)
                                    segs.append((s0, s1))
                                    s0 = s1
                                for t, (dr, dj) in enumerate(TAPS):
                                    delta = T * dr + dj
                                    dmat = diag_t[:, (br * 9 + t) * 128:(br * 9 + t + 1) * 128]
                                    for (s0, s1) in segs:
                                        nc.tensor.matmul(
                                            lp[:, s0:s1],
                                            dmat,
                                            vpad[:, s0 + delta:s1 + delta],
                                            start=(t == 0),
                                            stop=(not with_cbias and t == 8),
                                            skip_group_check=True)
                                if with_cbias:
                                    for (s0, s1) in segs:  # conv bias
                                        nc.tensor.matmul(
                                            lp[:, s0:s1],
                                            cbdiag[:, 128 * br:128 * (br + 1)],
                                            ones512[:, 0:s1 - s0],
                                            start=False, stop=True, skip_group_check=True)

                                # --- normalize + lepe -> concat ---
                                # rd[0:64] = 1/den rows 64:128; cat = att*rd + lepe
                                rd = stg.tile([128, 1024], F32, tag="recip", name="rd")
                                nc.vector.reciprocal_approx_fast(rd, avden)
                                catw = winview(cat_t[br], combo)[:, w]       # [c, R, J]
                                lpv = lp[:, RB:RB + span].rearrange(
                                    "c (r t) -> c r t", t=T)[:, :, 0:J]      # [c, R, J]
                                for hp in range(2):
                                    nc.vector.tensor_mul(
                                        rd[0:64, 512 * hp:512 * hp + 512],
                                        avden[0:64, 512 * hp:512 * hp + 512],
                                        rd[64:128, 512 * hp:512 * hp + 512])
                                for hp in range(2):
                                    nc.vector.tensor_add(
                                        catw[64 * hp:64 * hp + 64],
                                        rd[0:64, 512 * hp:512 * hp + 512].rearrange(
                                            "c (r j) -> c r j", j=J),
                                        lpv[64 * hp:64 * hp + 64])

                                if is_dbg:
                                    for nm, src_t, ln in [("d_av", avden, 1024),
                                                          ("d_rd", rd, 1024),
                                                          ("d_lep", lp, TOT)]:
                                        d = stg.tile([128, 1024], F32, tag="dbg" + nm, name="dT")
                                        nc.vector.tensor_copy(d[:, :ln], src_t[:, :ln])
                                        nc.sync.dma_start(dbg[nm][:, :ln], d[:, :ln])
                                    d1 = stg.tile([128, 2048], F32, tag="dbgexp", name="dexp")
                                    nc.vector.tensor_copy(d1[:, :1024], exp_tiles[(0, 0)])
                                    nc.vector.tensor_copy(d1[:, 1024:], exp_tiles[(1, 0)])
                                    nc.sync.dma_start(dbg["d_exp"], d1)
                                    dc = stg.tile([128, 512], F32, tag="dbgcat", name="dcat")
                                    nc.vector.tensor_copy(
                                        dc.rearrange("c (a b) -> c a b", a=R, b=J), catw)
                                    nc.sync.dma_start(dbg["d_cat"], dc)

                    # ---- proj tail ----
                    for n in range(32):
                        pp = auxps.tile([128, 256], F32, tag="aux", name="pp")
                        for b2 in range(4):
                            nc.tensor.matmul(pp, cat_t[b2][:, 128 * n:128 * (n + 1)],
                                             pw_t[b2], start=(b2 == 0), stop=(b2 == 3),
                                             skip_group_check=True)
                        osb = outsb.tile([128, 256], F32, tag="out", name="osb")
                        nc.vector.tensor_add(osb, pp, pb_t)
                        nc.sync.dma_start(out_d[128 * n:128 * (n + 1), :], osb)

            if dyn_loop:
                with tc.For_i(0, dyn_loop, 1):
                    _emit(0)
            else:
                for _rep in range(repeat):
                    _emit(_rep)

    return nc


_CACHE = {}


def _get_nc(debug=False, repeat=1, dyn_loop=0, with_cbias=True):
    key = (bool(debug), repeat, dyn_loop, with_cbias)
    if key not in _CACHE:
        nc = bacc.Bacc("TRN2", target_bir_lowering=False, debug=False)
        build(nc, debug=debug, repeat=repeat, dyn_loop=dyn_loop, with_cbias=with_cbias)
        nc.compile()
        _CACHE[key] = nc
    return _CACHE[key]


def prep_inputs(x, qkv_w, proj_w, proj_b, conv_ws, conv_bs):
    x = np.asarray(x)
    B = x.shape[0]
    xb = x.astype(ml_dtypes.bfloat16)
    qwb = np.asarray(qkv_w).astype(ml_dtypes.bfloat16)
    pwb = np.asarray(proj_w).astype(ml_dtypes.bfloat16)
    w9 = np.asarray(conv_ws).reshape(4, 128, 9).astype(np.float32)
    dgn = np.zeros((36, 128, 128), np.float32)
    idx = np.arange(128)
    for br in range(4):
        for t, (dr, dj) in enumerate(TAPS):
            dgn[br * 9 + t, idx, idx] = w9[br, :, (dr + 1) * 3 + (dj + 1)]
    dgn = dgn.astype(ml_dtypes.bfloat16)
    pbb = np.tile(np.asarray(proj_b, np.float32)[None, :], (128, 1))
    cbt = np.ascontiguousarray(np.asarray(conv_bs, np.float32).T)
    shared = {"qw": qwb, "pw": pwb, "dg": dgn, "pb": pbb, "cb": cbt}
    return [dict(shared, xb=np.ascontiguousarray(xb[b])) for b in range(B)]


def kernel(x, qkv_w, proj_w, proj_b, conv_ws, conv_bs, _debug=False, _trace=False):
    wcb = bool(np.any(np.asarray(conv_bs)))
    nc = _get_nc(debug=_debug, with_cbias=wcb)
    in_maps = prep_inputs(x, qkv_w, proj_w, proj_b, conv_ws, conv_bs)
    res = run_bass_kernel_spmd(nc, in_maps, core_ids=list(range(len(in_maps))),
                               trace=_trace)
    out = np.stack([r["out"] for r in res.results]).astype(np.float32)
    if _debug or _trace:
        kernel.last_results = res
    return out


# revision 15
# speedup vs baseline: 1.3814x; 1.1576x over previous
"""CSWin-style cross-attention block for Trainium2 (Bass/Tile), 8-core data-parallel.

Per core (one batch image, L=4096=64x64, C=256):
  qkv = x @ qkv_w; 4 branch attentions on half-channels with strip windows
  (64x8 / 8x64), depthwise-conv LePE added to attention out; concat; proj.

v3 mapping. ACT-exp is the bottleneck (measured: exp[128,1536] PSUM->SBUF,
PE-fed, back-to-back = ~1435 ns => 7.65 us per branch-window, 245 us/core):
  - continuous exp stream: score blocks ([128,512] S^T chunks) are written
    round-robin into 2 PSUM tiles of [128,1536] (3 banks each); each full
    tile fires ONE exp ACTIVATE (N=1536, crossing kc/branch/window
    boundaries). Larger activations amortize the ~350-cyc ACT overhead.
  - PSUM banks (8): sc 2x3 + avden 1 + lepe/aux 1.
  - AV 4-way col-tiled per (kc, head-pair): att (lhsT=V^T slice, M=32) and
    den (lhsT=ones, M=32) at array cols 0/32/64/96; avden [128,512] holds
    [att_h|att_h'|den_h|den_h'] for ONE head-pair; head-pairs sequential.
  - normalize per head-pair: recip [128,512] + one mul [64,512]; cat add
    fused with LePE readout per (hp, window-half).
  - LePE per half-window in a 1-bank PSUM tile: 9 diagonal-weight matmuls
    over a zero-padded flat layout (pitch J+1; shared pad col covers dj=+-1;
    cross-half taps read true neighbor rows from the full vpad).
  - window staging (combo A contiguous copies + V^T transposes) prefetched
    one window ahead so scores never wait on DVE at window boundaries.
  - proj per 128-token chunk: 4 branch matmuls accumulated in PSUM + bias.
"""
import os
import sys

sys.path.insert(0, "/opt/trn_rl_repo")
import numpy as np
import ml_dtypes

import concourse.bacc as bacc
import concourse.mybir as mybir
import concourse.tile as tile
from concourse.bass_utils import run_bass_kernel_spmd
from concourse.masks import make_identity

BF = mybir.dt.bfloat16
F32 = mybir.dt.float32
AF = mybir.ActivationFunctionType
ALU = mybir.AluOpType
SCALE = float(32.0 ** -0.5)

# tap order: (0,0) first so the start=True matmul covers the whole region
TAPS = [(0, 0)] + [(dr, dj) for dr in (-1, 0, 1) for dj in (-1, 0, 1) if (dr, dj) != (0, 0)]

# branch -> (combo, qhalf, kvhalf); combo A = 64x8 windows, B = 8x64
BRANCH = {0: ("A", 0, 0), 1: ("B", 1, 1), 2: ("A", 1, 0), 3: ("B", 0, 1)}

# padded flat window layouts for LePE: (rows, cols, row_pitch, region_base, total)
# pitch = cols+1: single shared pad col between consecutive rows covers dj=+-1.
PAD = {"A": (64, 8, 9, 16, 608), "B": (8, 64, 65, 68, 656)}
# half-window split along rows for the 1-bank lp tiles
NHALF = 2


class ExpStream:
    """Round-robin score blocks into [128,1536] PSUM tiles; one exp per tile."""

    def __init__(self, nc, scps, expsb, width=1536):
        self.nc = nc
        self.scps = scps
        self.expsb = expsb
        self.width = width
        self.nslot = width // 512
        self.cur = None
        self.slot = 0
        self.pending = []
        self.out = {}

    def add_block(self, key, emit_mms):
        if self.cur is None:
            self.cur = self.scps.tile([128, self.width], F32, tag="sc", name="sct")
            self.slot = 0
            self.pending = []
        dst = self.cur[:, 512 * self.slot:512 * self.slot + 512]
        emit_mms(dst)
        self.pending.append(key)
        self.slot += 1
        if self.slot == self.nslot:
            self.flush()

    def flush(self):
        if self.cur is None or self.slot == 0:
            return
        n = 512 * self.slot
        e = self.expsb.tile([128, self.width], BF, tag="exp", name="exp")
        self.nc.scalar.activation(e[:, :n], self.cur[:, :n], AF.Exp, scale=SCALE)
        for i, k in enumerate(self.pending):
            self.out[k] = e[:, 512 * i:512 * (i + 1)]
        self.cur = None
        self.slot = 0
        self.pending = []


def build(nc, debug=False, repeat=1, dyn_loop=0, with_cbias=True):
    xb = nc.dram_tensor("xb", [4096, 256], BF, kind="ExternalInput").ap()
    qw = nc.dram_tensor("qw", [256, 768], BF, kind="ExternalInput").ap()
    pw = nc.dram_tensor("pw", [512, 256], BF, kind="ExternalInput").ap()
    dg = nc.dram_tensor("dg", [36, 128, 128], BF, kind="ExternalInput").ap()
    pb = nc.dram_tensor("pb", [128, 256], F32, kind="ExternalInput").ap()
    cb = nc.dram_tensor("cb", [128, 4], F32, kind="ExternalInput").ap()
    out_d = nc.dram_tensor("out", [4096, 256], F32, kind="ExternalOutput").ap()
    dbg = {}
    if debug:
        for name, shape in [("d_av", [128, 512]), ("d_rd", [128, 512]),
                            ("d_lep", [128, 656]), ("d_cat", [128, 512]),
                            ("d_vt", [128, 512])]:
            dbg[name] = nc.dram_tensor(name, shape, F32, kind="ExternalOutput").ap()

    with tile.TileContext(nc) as tc:
        with tc.sbuf_pool(name="persist", bufs=1) as ps_pool:
            # ---- constants / weights ----
            ident = ps_pool.tile([128, 128], BF, name="ident")
            make_identity(nc, ident)
            ones32 = ps_pool.tile([128, 32], BF, name="ones32")
            nc.vector.memset(ones32, 1.0)
            ones512 = ps_pool.tile([128, 512], BF, name="ones512")
            nc.vector.memset(ones512, 1.0)
            cbdiag = ps_pool.tile([128, 4 * 128], BF, name="cbdiag")

            qw_t = [ps_pool.tile([128, 768], BF, name=f"qw{i}") for i in range(2)]
            for i in range(2):
                nc.sync.dma_start(qw_t[i], qw[128 * i:128 * (i + 1), :])
            pw_t = [ps_pool.tile([128, 256], BF, name=f"pw{i}") for i in range(4)]
            for i in range(4):
                nc.sync.dma_start(pw_t[i], pw[128 * i:128 * (i + 1), :])
            diag_t = ps_pool.tile([128, 36 * 128], BF, name="diag_t")
            nc.sync.dma_start(diag_t.rearrange("p (t c) -> p t c", t=36),
                              dg.rearrange("t p c -> p t c"))
            pb_t = ps_pool.tile([128, 256], F32, name="pb_t")
            nc.sync.dma_start(pb_t, pb)
            cb_t = ps_pool.tile([128, 4], F32, name="cb_t")
            nc.sync.dma_start(cb_t, cb)
            for _b in range(4):
                nc.vector.tensor_scalar(cbdiag[:, 128 * _b:128 * (_b + 1)], ident,
                                        cb_t[:, _b:_b + 1], None, ALU.mult)

            # ---- persistent activations ----
            q_t = [ps_pool.tile([128, 4096], BF, name=f"q{i}") for i in range(2)]
            k_t = [ps_pool.tile([128, 4096], BF, name=f"k{i}") for i in range(2)]
            v_t = [ps_pool.tile([128, 4096], BF, name=f"v{i}") for i in range(2)]
            cat_t = [ps_pool.tile([128, 4096], BF, name=f"cat{i}") for i in range(4)]

            # persistent zero-padded LePE staging (borders zeroed once,
            # interiors rewritten per window; 2 bufs each for overlap)
            vpads = {}
            for combo in ("A", "B"):
                R, J, T, RB, TOT = PAD[combo]
                tiles = [ps_pool.tile([128, TOT], BF, name=f"vpad{combo}{i}")
                         for i in range(3)]
                for t in tiles:
                    nc.vector.memset(t, 0.0)
                vpads[combo] = tiles

            def _emit(_rep):
                # ================= phase 0: x^T + QKV =================
                with tc.sbuf_pool(name=f"p0sb{_rep}", bufs=1) as p0sb, \
                     tc.tile_pool(name=f"p0ps{_rep}", bufs=4, space="PSUM") as p0ps, \
                     tc.tile_pool(name=f"p0ps2{_rep}", bufs=4, space="PSUM") as p0ps2, \
                     tc.sbuf_pool(name=f"p0in{_rep}", bufs=4) as p0in:
                    xT = [p0sb.tile([128, 4096], BF, name=f"xT{i}") for i in range(2)]
                    for n in range(32):
                        xin = p0in.tile([128, 256], BF, tag="xin", name="xin")
                        nc.sync.dma_start(xin, xb[128 * n:128 * (n + 1), :])
                        for cc in range(2):
                            tp = p0ps.tile([128, 128], BF, tag="tp", name="tp")
                            nc.tensor.transpose(tp, xin[:, 128 * cc:128 * (cc + 1)], ident)
                            nc.vector.tensor_copy(xT[cc][:, 128 * n:128 * (n + 1)], tp)
                    for n in range(8):
                        for m in range(6):
                            qp = p0ps2.tile([128, 512], F32, tag="qp", name="qp")
                            for cc in range(2):
                                nc.tensor.matmul(qp, qw_t[cc][:, 128 * m:128 * (m + 1)],
                                                 xT[cc][:, 512 * n:512 * (n + 1)],
                                                 start=(cc == 0), stop=(cc == 1),
                                                 skip_group_check=True)
                            dst = [q_t, k_t, v_t][m // 2][m % 2]
                            # split evacuation between ACT (idle in phase 0) and DVE
                            if m % 2 == 0:
                                nc.scalar.copy(dst[:, 512 * n:512 * (n + 1)], qp)
                            else:
                                nc.vector.tensor_copy(dst[:, 512 * n:512 * (n + 1)], qp)

                # window views (for DVE staging copies only; matmuls need 1-D free)
                def winview(t, combo):
                    if combo == "A":
                        return t.rearrange("c (r w j) -> c w r j", r=64, w=8, j=8)
                    return t.rearrange("c (w i cc) -> c w i cc", w=8, i=8, cc=64)

                # ================= attention =================
                with tc.tile_pool(name=f"scps{_rep}", bufs=2, space="PSUM") as scps, \
                     tc.tile_pool(name=f"avps{_rep}", bufs=1, space="PSUM") as avps, \
                     tc.tile_pool(name=f"auxps{_rep}", bufs=1, space="PSUM") as auxps, \
                     tc.sbuf_pool(name=f"expsb{_rep}", bufs=13) as expsb, \
                     tc.sbuf_pool(name=f"stg{_rep}", bufs=3) as stg, \
                     tc.sbuf_pool(name=f"rdsb{_rep}", bufs=3) as rdsb, \
                     tc.sbuf_pool(name=f"outsb{_rep}", bufs=4) as outsb:

                    es = ExpStream(nc, scps, expsb)

                    def stage_window(combo, w):
                        """Stage contiguous q/k/v windows + V^T for (combo, w)."""
                        R, J, T, RB, TOT = PAD[combo]
                        branches = (0, 2) if combo == "A" else (1, 3)
                        kvhalf = BRANCH[branches[0]][2]
                        if combo == "A":
                            kwin = stg.tile([128, 512], BF, tag="kwin", name="kwin")
                            nc.vector.tensor_copy(
                                kwin.rearrange("c (r j) -> c r j", j=8),
                                winview(k_t[kvhalf], "A")[:, w])
                            vwin = stg.tile([128, 512], BF, tag="vwin", name="vwin")
                            nc.vector.tensor_copy(
                                vwin.rearrange("c (r j) -> c r j", j=8),
                                winview(v_t[kvhalf], "A")[:, w])
                            qwin = {}
                            for qh in (0, 1):
                                qt = stg.tile([128, 512], BF, tag=f"qwin{qh}", name="qwin")
                                nc.vector.tensor_copy(
                                    qt.rearrange("c (r j) -> c r j", j=8),
                                    winview(q_t[qh], "A")[:, w])
                                qwin[qh] = qt
                        else:
                            kwin = k_t[kvhalf][:, 512 * w:512 * (w + 1)]
                            vwin = v_t[kvhalf][:, 512 * w:512 * (w + 1)]
                            qwin = {qh: q_t[qh][:, 512 * w:512 * (w + 1)]
                                    for qh in (0, 1)}
                        # zero-padded v window for LePE (interior only)
                        vpad = vpads[combo][w % 3]
                        nc.vector.tensor_copy(
                            vpad[:, RB:RB + R * T].rearrange(
                                "c (r t) -> c r t", t=T)[:, :, 0:J],
                            vwin.rearrange("c (r j) -> c r j", j=J))
                        # V^T: 4 PE transposes (aux psum bank) + one copy
                        vtp = auxps.tile([128, 512], BF, tag="aux", name="vtp")
                        for kc in range(4):
                            nc.tensor.transpose(vtp[:, 128 * kc:128 * (kc + 1)],
                                                vwin[:, 128 * kc:128 * (kc + 1)], ident)
                        vt_sb = stg.tile([128, 512], BF, tag="vt", name="vt_sb")
                        nc.vector.tensor_copy(vt_sb, vtp)
                        return dict(kwin=kwin, vwin=vwin, qwin=qwin, vpad=vpad,
                                    vt=vt_sb)

                    def emit_scores(combo, w, br, st):
                        """Feed this branch-window's 16 score blocks into the
                        exp stream; actual AV/norm/LePE runs one bw later."""
                        _, qhalf, kvh = BRANCH[br]
                        qfull = st["qwin"][qhalf]
                        kwin = st["kwin"]

                        def mk_mm(h, kc):
                            def emit(dst):
                                nc.tensor.matmul(
                                    dst,
                                    kwin[32 * h:32 * (h + 1), 128 * kc:128 * (kc + 1)],
                                    qfull[32 * h:32 * (h + 1), :],
                                    start=True, stop=True,
                                    tile_position=(32 * h, 0))
                            return emit

                        for kc in range(4):
                            for h in range(4):
                                es.add_block((br, w, h, kc), mk_mm(h, kc))
                        return (combo, w, br, st)

                    def consume(ctx):
                        combo, w, br, st = ctx
                        R, J, T, RB, TOT = PAD[combo]
                        vt_sb, vpad = st["vt"], st["vpad"]
                        is_dbg = debug and br == 0 and w == 0
                        span = R * T // NHALF
                        rh = R // NHALF

                        def lepe_half(half):
                            base = RB + span * half
                            lp = auxps.tile([128, RB + span], F32, tag="aux", name="lp")
                            for t, (dr, dj) in enumerate(TAPS):
                                delta = T * dr + dj
                                dmat = diag_t[:, (br * 9 + t) * 128:(br * 9 + t + 1) * 128]
                                nc.tensor.matmul(
                                    lp[:, RB:RB + span],
                                    dmat,
                                    vpad[:, base + delta:base + span + delta],
                                    start=(t == 0),
                                    stop=(not with_cbias and t == 8),
                                    skip_group_check=True)
                            if with_cbias:
                                nc.tensor.matmul(
                                    lp[:, RB:RB + span],
                                    cbdiag[:, 128 * br:128 * (br + 1)],
                                    ones512[:, 0:span],
                                    start=False, stop=True, skip_group_check=True)
                            return lp

                        # --- AV + den per head-pair (sequential, 1-bank avden) ---
                        rds = []
                        for hp in range(2):
                            avden = avps.tile([128, 512], F32, tag="av", name="avden")
                            for kc in range(4):
                                for hs in range(2):
                                    h = 2 * hp + hs
                                    ecols = es.out[(br, w, h, kc)]
                                    nc.tensor.matmul(
                                        avden[32 * hs:32 * hs + 32, :],
                                        vt_sb[:, 128 * kc + 32 * h:128 * kc + 32 * h + 32],
                                        ecols,
                                        start=(kc == 0), stop=(kc == 3),
                                        tile_position=(0, 32 * hs),
                                        skip_group_check=True)
                                    nc.tensor.matmul(
                                        avden[64 + 32 * hs:64 + 32 * hs + 32, :],
                                        ones32,
                                        ecols,
                                        start=(kc == 0), stop=(kc == 3),
                                        tile_position=(0, 64 + 32 * hs),
                                        skip_group_check=True)
                            # normalize: rd[64:128]=1/den (full-tile custom op
                            # reads base partition 0); rd[0:64]=att*recip
                            rd = rdsb.tile([128, 512], F32, tag="rd", name="rd")
                            nc.vector.reciprocal_approx_fast(rd, avden)
                            nc.vector.tensor_mul(rd[0:64, :], avden[0:64, :],
                                                 rd[64:128, :])
                            rds.append(rd)
                            if is_dbg and hp == 0:
                                for nm, src in [("d_av", avden), ("d_rd", rd)]:
                                    dt_ = stg.tile([128, 512], F32, tag="dbg" + nm, name="dT")
                                    nc.vector.tensor_copy(dt_, src)
                                    nc.sync.dma_start(dbg[nm], dt_)

                        # --- cat = att*rd + lepe, per (hp, half); lp halves
                        # sequential on the single aux bank: adds(h) must
                        # drain before taps(h+1) ---
                        catw = winview(cat_t[br], combo)[:, w]       # [c, R, J]
                        tviews = [rds[hp][0:64, :].rearrange(
                            "c (r j) -> c r j", j=J) for hp in range(2)]

                        def add_half(half, lp):
                            lpv = lp[:, RB:RB + span].rearrange(
                                "c (r t) -> c r t", t=T)[:, :, 0:J]
                            for hp in range(2):
                                nc.vector.tensor_add(
                                    catw[64 * hp:64 * hp + 64,
                                         rh * half:rh * (half + 1)],
                                    tviews[hp][:, rh * half:rh * (half + 1)],
                                    lpv[64 * hp:64 * hp + 64])
                            if is_dbg:
                                dl = stg.tile([128, 328], F32, tag="dbglep", name="dlep")
                                nc.vector.tensor_copy(dl[:, :RB + span], lp)
                                nc.sync.dma_start(
                                    dbg["d_lep"][:, 328 * half:328 * half + RB + span],
                                    dl[:, :RB + span])

                        for half in range(NHALF):
                            add_half(half, lepe_half(half))

                        if is_dbg:
                            dc = stg.tile([128, 512], F32, tag="dbgcat", name="dcat")
                            nc.vector.tensor_copy(
                                dc.rearrange("c (a b) -> c a b", a=R, b=J), catw)
                            nc.sync.dma_start(dbg["d_cat"], dc)
                            dv = stg.tile([128, 512], F32, tag="dbgvt", name="dvt")
                            nc.vector.tensor_copy(dv, vt_sb)
                            nc.sync.dma_start(dbg["d_vt"], dv)

                    # window order: all B windows, then all A windows; staging
                    # prefetched one window ahead; AV/norm/LePE consumption
                    # deferred one branch-window so its exps have flushed
                    order = [("B", w) for w in range(8)] + [("A", w) for w in range(8)]
                    staged = {}
                    staged[order[0]] = stage_window(*order[0])
                    pending = None
                    for i, (combo, w) in enumerate(order):
                        if i + 1 < len(order):
                            staged[order[i + 1]] = stage_window(*order[i + 1])
                        st = staged.pop((combo, w))
                        branches = (0, 2) if combo == "A" else (1, 3)
                        for br in branches:
                            ctx = emit_scores(combo, w, br, st)
                            if pending is not None:
                                consume(pending)
                            pending = ctx
                    es.flush()
                    consume(pending)

                    # ---- proj tail ----
                    for n in range(32):
                        pp = auxps.tile([128, 256], F32, tag="aux", name="pp")
                        for b2 in range(4):
                            nc.tensor.matmul(pp, cat_t[b2][:, 128 * n:128 * (n + 1)],
                                             pw_t[b2], start=(b2 == 0), stop=(b2 == 3),
                                             skip_group_check=True)
                        osb = outsb.tile([128, 256], F32, tag="out", name="osb")
                        nc.vector.tensor_add(osb, pp, pb_t)
                        nc.sync.dma_start(out_d[128 * n:128 * (n + 1), :], osb)

            if dyn_loop:
                with tc.For_i(0, dyn_loop, 1):
                    _emit(0)
            else:
                for _rep in range(repeat):
                    _emit(_rep)

    return nc


_CACHE = {}


def _get_nc(debug=False, repeat=1, dyn_loop=0, with_cbias=True):
    key = (bool(debug), repeat, dyn_loop, with_cbias)
    if key not in _CACHE:
        nc = bacc.Bacc("TRN2", target_bir_lowering=False, debug=False)
        build(nc, debug=debug, repeat=repeat, dyn_loop=dyn_loop, with_cbias=with_cbias)
        nc.compile()
        _CACHE[key] = nc
    return _CACHE[key]


def prep_inputs(x, qkv_w, proj_w, proj_b, conv_ws, conv_bs):
    x = np.asarray(x)
    B = x.shape[0]
    xb = x.astype(ml_dtypes.bfloat16)
    qwb = np.asarray(qkv_w).astype(ml_dtypes.bfloat16)
    pwb = np.asarray(proj_w).astype(ml_dtypes.bfloat16)
    w9 = np.asarray(conv_ws).reshape(4, 128, 9).astype(np.float32)
    dgn = np.zeros((36, 128, 128), np.float32)
    idx = np.arange(128)
    for br in range(4):
        for t, (dr, dj) in enumerate(TAPS):
            dgn[br * 9 + t, idx, idx] = w9[br, :, (dr + 1) * 3 + (dj + 1)]
    dgn = dgn.astype(ml_dtypes.bfloat16)
    pbb = np.tile(np.asarray(proj_b, np.float32)[None, :], (128, 1))
    cbt = np.ascontiguousarray(np.asarray(conv_bs, np.float32).T)
    shared = {"qw": qwb, "pw": pwb, "dg": dgn, "pb": pbb, "cb": cbt}
    return [dict(shared, xb=np.ascontiguousarray(xb[b])) for b in range(B)]


def kernel(x, qkv_w, proj_w, proj_b, conv_ws, conv_bs, _debug=False, _trace=False):
    wcb = bool(np.any(np.asarray(conv_bs)))
    nc = _get_nc(debug=_debug, with_cbias=wcb)
    in_maps = prep_inputs(x, qkv_w, proj_w, proj_b, conv_ws, conv_bs)
    res = run_bass_kernel_spmd(nc, in_maps, core_ids=list(range(len(in_maps))),
                               trace=_trace)
    out = np.stack([r["out"] for r in res.results]).astype(np.float32)
    if _debug or _trace:
        kernel.last_results = res
    return out


# revision 21
# speedup vs baseline: 7.6316x; 5.5244x over previous
"""CSWin-style cross-attention block for Trainium2 (Bass/Tile), 8-core data-parallel.

Per core (one batch image, L=4096=64x64, C=256):
  qkv = x @ qkv_w; 4 branch attentions on half-channels with strip windows
  (64x8 / 8x64), depthwise-conv LePE added to attention out; concat; proj.

v3 mapping. ACT-exp is the bottleneck (measured: exp[128,1536] PSUM->SBUF,
PE-fed, back-to-back = ~1435 ns => 7.65 us per branch-window, 245 us/core):
  - continuous exp stream: score blocks ([128,512] S^T chunks) are written
    round-robin into 2 PSUM tiles of [128,1536] (3 banks each); each full
    tile fires ONE exp ACTIVATE (N=1536, crossing kc/branch/window
    boundaries). Larger activations amortize the ~350-cyc ACT overhead.
  - PSUM banks (8): sc 2x3 + avden 1 + lepe/aux 1.
  - AV 4-way col-tiled per (kc, head-pair): att (lhsT=V^T slice, M=32) and
    den (lhsT=ones, M=32) at array cols 0/32/64/96; avden [128,512] holds
    [att_h|att_h'|den_h|den_h'] for ONE head-pair; head-pairs sequential.
  - normalize per head-pair: recip [128,512] + one mul [64,512]; cat add
    fused with LePE readout per (hp, window-half).
  - LePE per half-window in a 1-bank PSUM tile: 9 diagonal-weight matmuls
    over a zero-padded flat layout (pitch J+1; shared pad col covers dj=+-1;
    cross-half taps read true neighbor rows from the full vpad).
  - window staging (combo A contiguous copies + V^T transposes) prefetched
    one window ahead so scores never wait on DVE at window boundaries.
  - proj per 128-token chunk: 4 branch matmuls accumulated in PSUM + bias.
"""
import os
import sys

sys.path.insert(0, "/opt/trn_rl_repo")
import numpy as np
import ml_dtypes

import concourse.bacc as bacc
import concourse.mybir as mybir
import concourse.tile as tile
from concourse.bass_utils import run_bass_kernel_spmd
from concourse.masks import make_identity

BF = mybir.dt.bfloat16
F32 = mybir.dt.float32
AF = mybir.ActivationFunctionType
ALU = mybir.AluOpType
SCALE = float(32.0 ** -0.5)

# tap order: (0,0) first so the start=True matmul covers the whole region
TAPS = [(0, 0)] + [(dr, dj) for dr in (-1, 0, 1) for dj in (-1, 0, 1) if (dr, dj) != (0, 0)]

# branch -> (combo, qhalf, kvhalf); combo A = 64x8 windows, B = 8x64
BRANCH = {0: ("A", 0, 0), 1: ("B", 1, 1), 2: ("A", 1, 0), 3: ("B", 0, 1)}

# padded flat window layouts for LePE: (rows, cols, row_pitch, region_base, total)
# pitch = cols+1: single shared pad col between consecutive rows covers dj=+-1.
PAD = {"A": (64, 8, 9, 16, 608), "B": (8, 64, 65, 68, 656)}
# half-window split along rows for the 1-bank lp tiles
NHALF = 2


class ExpStream:
    """Round-robin score blocks into [128,1536] PSUM tiles; one exp per tile."""

    def __init__(self, nc, scps, expsb, width=1536):
        self.nc = nc
        self.scps = scps
        self.expsb = expsb
        self.width = width
        self.nslot = width // 512
        self.cur = None
        self.slot = 0
        self.pending = []
        self.out = {}
        self.on_flush = None

    def add_block(self, key, emit_mms):
        if self.cur is None:
            self.cur = self.scps.tile([128, self.width], F32, tag="sc", name="sct")
            self.slot = 0
            self.pending = []
        dst = self.cur[:, 512 * self.slot:512 * self.slot + 512]
        emit_mms(dst)
        self.pending.append(key)
        self.slot += 1
        if self.slot == self.nslot:
            self.flush()

    def flush(self):
        if self.cur is None or self.slot == 0:
            return
        n = 512 * self.slot
        e = self.expsb.tile([128, self.width], BF, tag="exp", name="exp")
        self.nc.scalar.activation(e[:, :n], self.cur[:, :n], AF.Exp, scale=SCALE)
        for i, k in enumerate(self.pending):
            self.out[k] = e[:, 512 * i:512 * (i + 1)]
        self.cur = None
        self.slot = 0
        self.pending = []
        if self.on_flush is not None:
            self.on_flush()


def build(nc, debug=False, repeat=1, dyn_loop=0, with_cbias=True):
    xb = nc.dram_tensor("xb", [4096, 256], BF, kind="ExternalInput").ap()
    qw = nc.dram_tensor("qw", [256, 768], BF, kind="ExternalInput").ap()
    pw = nc.dram_tensor("pw", [512, 256], BF, kind="ExternalInput").ap()
    dg = nc.dram_tensor("dg", [36, 128, 128], BF, kind="ExternalInput").ap()
    pb = nc.dram_tensor("pb", [128, 256], F32, kind="ExternalInput").ap()
    cb = nc.dram_tensor("cb", [128, 4], F32, kind="ExternalInput").ap()
    out_d = nc.dram_tensor("out", [4096, 256], F32, kind="ExternalOutput").ap()
    dbg = {}
    if debug:
        for name, shape in [("d_av", [128, 512]), ("d_rd", [128, 512]),
                            ("d_lep", [128, 656]), ("d_cat", [128, 512]),
                            ("d_vt", [128, 512])]:
            dbg[name] = nc.dram_tensor(name, shape, F32, kind="ExternalOutput").ap()

    with tile.TileContext(nc) as tc:
        with tc.sbuf_pool(name="persist", bufs=1) as ps_pool:
            # ---- constants / weights ----
            ident = ps_pool.tile([128, 128], BF, name="ident")
            make_identity(nc, ident)
            ones32 = ps_pool.tile([128, 32], BF, name="ones32")
            nc.vector.memset(ones32, 1.0)
            ones512 = ps_pool.tile([128, 512], BF, name="ones512")
            nc.vector.memset(ones512, 1.0)
            cbdiag = ps_pool.tile([128, 4 * 128], BF, name="cbdiag")

            qw_t = [ps_pool.tile([128, 768], BF, name=f"qw{i}") for i in range(2)]
            for i in range(2):
                nc.sync.dma_start(qw_t[i], qw[128 * i:128 * (i + 1), :])
            pw_t = [ps_pool.tile([128, 256], BF, name=f"pw{i}") for i in range(4)]
            for i in range(4):
                nc.sync.dma_start(pw_t[i], pw[128 * i:128 * (i + 1), :])
            diag_t = ps_pool.tile([128, 36 * 128], BF, name="diag_t")
            nc.sync.dma_start(diag_t.rearrange("p (t c) -> p t c", t=36),
                              dg.rearrange("t p c -> p t c"))
            pb_t = ps_pool.tile([128, 256], F32, name="pb_t")
            nc.sync.dma_start(pb_t, pb)
            cb_t = ps_pool.tile([128, 4], F32, name="cb_t")
            nc.sync.dma_start(cb_t, cb)
            for _b in range(4):
                nc.vector.tensor_scalar(cbdiag[:, 128 * _b:128 * (_b + 1)], ident,
                                        cb_t[:, _b:_b + 1], None, ALU.mult)

            # ---- persistent activations ----
            q_t = [ps_pool.tile([128, 4096], BF, name=f"q{i}") for i in range(2)]
            k_t = [ps_pool.tile([128, 4096], BF, name=f"k{i}") for i in range(2)]
            v_t = [ps_pool.tile([128, 4096], BF, name=f"v{i}") for i in range(2)]
            cat_t = [ps_pool.tile([128, 4096], BF, name=f"cat{i}") for i in range(4)]

            # persistent zero-padded LePE staging (borders zeroed once,
            # interiors rewritten per window; 2 bufs each for overlap)
            vpads = {}
            for combo in ("A", "B"):
                R, J, T, RB, TOT = PAD[combo]
                tiles = [ps_pool.tile([128, TOT], BF, name=f"vpad{combo}{i}")
                         for i in range(3)]
                for t in tiles:
                    nc.vector.memset(t, 0.0)
                vpads[combo] = tiles

            def _emit(_rep):
                # ================= phase 0: x^T + QKV =================
                with tc.sbuf_pool(name=f"p0sb{_rep}", bufs=1) as p0sb, \
                     tc.tile_pool(name=f"p0ps{_rep}", bufs=4, space="PSUM") as p0ps, \
                     tc.tile_pool(name=f"p0ps2{_rep}", bufs=4, space="PSUM") as p0ps2, \
                     tc.sbuf_pool(name=f"p0in{_rep}", bufs=4) as p0in:
                    xT = [p0sb.tile([128, 4096], BF, name=f"xT{i}") for i in range(2)]
                    for n in range(32):
                        xin = p0in.tile([128, 256], BF, tag="xin", name="xin")
                        nc.sync.dma_start(xin, xb[128 * n:128 * (n + 1), :])
                        for cc in range(2):
                            tp = p0ps.tile([128, 128], BF, tag="tp", name="tp")
                            nc.tensor.transpose(tp, xin[:, 128 * cc:128 * (cc + 1)], ident)
                            nc.vector.tensor_copy(xT[cc][:, 128 * n:128 * (n + 1)], tp)
                    for n in range(8):
                        for m in range(6):
                            qp = p0ps2.tile([128, 512], F32, tag="qp", name="qp")
                            for cc in range(2):
                                nc.tensor.matmul(qp, qw_t[cc][:, 128 * m:128 * (m + 1)],
                                                 xT[cc][:, 512 * n:512 * (n + 1)],
                                                 start=(cc == 0), stop=(cc == 1),
                                                 skip_group_check=True)
                            dst = [q_t, k_t, v_t][m // 2][m % 2]
                            # split evacuation between ACT (idle in phase 0) and DVE
                            if m % 2 == 0:
                                nc.scalar.copy(dst[:, 512 * n:512 * (n + 1)], qp)
                            else:
                                nc.vector.tensor_copy(dst[:, 512 * n:512 * (n + 1)], qp)

                # window views (for DVE staging copies only; matmuls need 1-D free)
                def winview(t, combo):
                    if combo == "A":
                        return t.rearrange("c (r w j) -> c w r j", r=64, w=8, j=8)
                    return t.rearrange("c (w i cc) -> c w i cc", w=8, i=8, cc=64)

                # ================= attention =================
                with tc.tile_pool(name=f"scps{_rep}", bufs=2, space="PSUM") as scps, \
                     tc.tile_pool(name=f"avps{_rep}", bufs=1, space="PSUM") as avps, \
                     tc.tile_pool(name=f"auxps{_rep}", bufs=1, space="PSUM") as auxps, \
                     tc.sbuf_pool(name=f"expsb{_rep}", bufs=13) as expsb, \
                     tc.sbuf_pool(name=f"stg{_rep}", bufs=3) as stg, \
                     tc.sbuf_pool(name=f"rdsb{_rep}", bufs=3) as rdsb, \
                     tc.sbuf_pool(name=f"outsb{_rep}", bufs=4) as outsb:

                    es = ExpStream(nc, scps, expsb)
                    # deferred work quanta (closures), popped one per exp-tile
                    # flush so PE's strict-FIFO order never head-of-line
                    # blocks score matmuls behind bulky AV/LePE work
                    workq = []

                    def pop_work():
                        if workq:
                            workq.pop(0)()

                    def stage_window(combo, w):
                        """Stage contiguous q/k/v windows + V^T for (combo, w)."""
                        R, J, T, RB, TOT = PAD[combo]
                        branches = (0, 2) if combo == "A" else (1, 3)
                        kvhalf = BRANCH[branches[0]][2]
                        if combo == "A":
                            kwin = stg.tile([128, 512], BF, tag="kwin", name="kwin")
                            nc.vector.tensor_copy(
                                kwin.rearrange("c (r j) -> c r j", j=8),
                                winview(k_t[kvhalf], "A")[:, w])
                            vwin = stg.tile([128, 512], BF, tag="vwin", name="vwin")
                            nc.vector.tensor_copy(
                                vwin.rearrange("c (r j) -> c r j", j=8),
                                winview(v_t[kvhalf], "A")[:, w])
                            qwin = {}
                            for qh in (0, 1):
                                qt = stg.tile([128, 512], BF, tag=f"qwin{qh}", name="qwin")
                                nc.vector.tensor_copy(
                                    qt.rearrange("c (r j) -> c r j", j=8),
                                    winview(q_t[qh], "A")[:, w])
                                qwin[qh] = qt
                        else:
                            kwin = k_t[kvhalf][:, 512 * w:512 * (w + 1)]
                            vwin = v_t[kvhalf][:, 512 * w:512 * (w + 1)]
                            qwin = {qh: q_t[qh][:, 512 * w:512 * (w + 1)]
                                    for qh in (0, 1)}
                        # zero-padded v window for LePE (interior only)
                        vpad = vpads[combo][w % 3]
                        nc.vector.tensor_copy(
                            vpad[:, RB:RB + R * T].rearrange(
                                "c (r t) -> c r t", t=T)[:, :, 0:J],
                            vwin.rearrange("c (r j) -> c r j", j=J))
                        # V^T: 4 PE transposes (aux psum bank) + one copy
                        vtp = auxps.tile([128, 512], BF, tag="aux", name="vtp")
                        for kc in range(4):
                            nc.tensor.transpose(vtp[:, 128 * kc:128 * (kc + 1)],
                                                vwin[:, 128 * kc:128 * (kc + 1)], ident)
                        vt_sb = stg.tile([128, 512], BF, tag="vt", name="vt_sb")
                        nc.vector.tensor_copy(vt_sb, vtp)
                        return dict(kwin=kwin, vwin=vwin, qwin=qwin, vpad=vpad,
                                    vt=vt_sb)

                    def emit_scores(combo, w, br, st):
                        """Feed this branch-window's 16 score blocks into the
                        exp stream; actual AV/norm/LePE runs one bw later."""
                        _, qhalf, kvh = BRANCH[br]
                        qfull = st["qwin"][qhalf]
                        kwin = st["kwin"]

                        def mk_mm(h, kc):
                            def emit(dst):
                                nc.tensor.matmul(
                                    dst,
                                    kwin[32 * h:32 * (h + 1), 128 * kc:128 * (kc + 1)],
                                    qfull[32 * h:32 * (h + 1), :],
                                    start=True, stop=True,
                                    tile_position=(32 * h, 0))
                            return emit

                        for kc in range(4):
                            for h in range(4):
                                es.add_block((br, w, h, kc), mk_mm(h, kc))
                        return (combo, w, br, st)

                    def consume(ctx):
                        combo, w, br, st = ctx
                        R, J, T, RB, TOT = PAD[combo]
                        vt_sb, vpad = st["vt"], st["vpad"]
                        is_dbg = debug and br == 0 and w == 0
                        span = R * T // NHALF
                        rh = R // NHALF

                        def lepe_half(half):
                            base = RB + span * half
                            lp = auxps.tile([128, RB + span], F32, tag="aux", name="lp")
                            for t, (dr, dj) in enumerate(TAPS):
                                delta = T * dr + dj
                                dmat = diag_t[:, (br * 9 + t) * 128:(br * 9 + t + 1) * 128]
                                nc.tensor.matmul(
                                    lp[:, RB:RB + span],
                                    dmat,
                                    vpad[:, base + delta:base + span + delta],
                                    start=(t == 0),
                                    stop=(not with_cbias and t == 8),
                                    skip_group_check=True)
                            if with_cbias:
                                nc.tensor.matmul(
                                    lp[:, RB:RB + span],
                                    cbdiag[:, 128 * br:128 * (br + 1)],
                                    ones512[:, 0:span],
                                    start=False, stop=True, skip_group_check=True)
                            return lp

                        # --- AV + den for one head-pair (1-bank avden) ---
                        rds = []

                        def av_norm(hp):
                            avden = avps.tile([128, 512], F32, tag="av", name="avden")
                            for kc in range(4):
                                for hs in range(2):
                                    h = 2 * hp + hs
                                    ecols = es.out[(br, w, h, kc)]
                                    nc.tensor.matmul(
                                        avden[32 * hs:32 * hs + 32, :],
                                        vt_sb[:, 128 * kc + 32 * h:128 * kc + 32 * h + 32],
                                        ecols,
                                        start=(kc == 0), stop=(kc == 3),
                                        tile_position=(0, 32 * hs),
                                        skip_group_check=True)
                                    nc.tensor.matmul(
                                        avden[64 + 32 * hs:64 + 32 * hs + 32, :],
                                        ones32,
                                        ecols,
                                        start=(kc == 0), stop=(kc == 3),
                                        tile_position=(0, 64 + 32 * hs),
                                        skip_group_check=True)
                            # normalize: rd[64:128]=1/den (full-tile custom op
                            # reads base partition 0); rd[0:64]=att*recip
                            rd = rdsb.tile([128, 512], F32, tag="rd", name="rd")
                            nc.vector.reciprocal_approx_fast(rd, avden)
                            nc.vector.tensor_mul(rd[0:64, :], avden[0:64, :],
                                                 rd[64:128, :])
                            rds.append(rd)
                            if is_dbg and hp == 0:
                                for nm, src in [("d_av", avden), ("d_rd", rd)]:
                                    dt_ = stg.tile([128, 512], F32, tag="dbg" + nm, name="dT")
                                    nc.vector.tensor_copy(dt_, src)
                                    nc.sync.dma_start(dbg[nm], dt_)

                        # --- cat = att*rd + lepe for one window-half; lp
                        # halves sequential on the single aux bank ---
                        catw = winview(cat_t[br], combo)[:, w]       # [c, R, J]

                        def lepe_add_half(half):
                            lp = lepe_half(half)
                            lpv = lp[:, RB:RB + span].rearrange(
                                "c (r t) -> c r t", t=T)[:, :, 0:J]
                            for hp in range(2):
                                tview = rds[hp][0:64, :].rearrange(
                                    "c (r j) -> c r j", j=J)
                                nc.vector.tensor_add(
                                    catw[64 * hp:64 * hp + 64,
                                         rh * half:rh * (half + 1)],
                                    tview[:, rh * half:rh * (half + 1)],
                                    lpv[64 * hp:64 * hp + 64])
                            if is_dbg:
                                dl = stg.tile([128, 328], F32, tag="dbglep", name="dlep")
                                nc.vector.tensor_copy(dl[:, :RB + span], lp)
                                nc.sync.dma_start(
                                    dbg["d_lep"][:, 328 * half:328 * half + RB + span],
                                    dl[:, :RB + span])
                            if is_dbg and half == NHALF - 1:
                                dc = stg.tile([128, 512], F32, tag="dbgcat", name="dcat")
                                nc.vector.tensor_copy(
                                    dc.rearrange("c (a b) -> c a b", a=R, b=J), catw)
                                nc.sync.dma_start(dbg["d_cat"], dc)
                                dv = stg.tile([128, 512], F32, tag="dbgvt", name="dvt")
                                nc.vector.tensor_copy(dv, vt_sb)
                                nc.sync.dma_start(dbg["d_vt"], dv)

                        workq.append(lambda: av_norm(0))
                        workq.append(lambda: av_norm(1))
                        workq.append(lambda: lepe_add_half(0))
                        workq.append(lambda: lepe_add_half(1))

                    # window order: all B windows, then all A windows; staging
                    # prefetched one window ahead; AV/norm/LePE consumption
                    # deferred one branch-window so its exps have flushed
                    order = [("B", w) for w in range(8)] + [("A", w) for w in range(8)]
                    es.on_flush = pop_work
                    staged = {}
                    staged[order[0]] = stage_window(*order[0])
                    for i, (combo, w) in enumerate(order):
                        if i + 1 < len(order):
                            staged[order[i + 1]] = stage_window(*order[i + 1])
                        st = staged.pop((combo, w))
                        branches = (0, 2) if combo == "A" else (1, 3)
                        for br in branches:
                            consume(emit_scores(combo, w, br, st))
                    es.flush()
                    es.on_flush = None
                    while workq:
                        pop_work()

                    # ---- proj tail ----
                    for n in range(32):
                        pp = auxps.tile([128, 256], F32, tag="aux", name="pp")
                        for b2 in range(4):
                            nc.tensor.matmul(pp, cat_t[b2][:, 128 * n:128 * (n + 1)],
                                             pw_t[b2], start=(b2 == 0), stop=(b2 == 3),
                                             skip_group_check=True)
                        osb = outsb.tile([128, 256], F32, tag="out", name="osb")
                        nc.vector.tensor_add(osb, pp, pb_t)
                        nc.sync.dma_start(out_d[128 * n:128 * (n + 1), :], osb)

            if dyn_loop:
                with tc.For_i(0, dyn_loop, 1):
                    _emit(0)
            else:
                for _rep in range(repeat):
                    _emit(_rep)

    return nc


_CACHE = {}


def _get_nc(debug=False, repeat=1, dyn_loop=0, with_cbias=True):
    key = (bool(debug), repeat, dyn_loop, with_cbias)
    if key not in _CACHE:
        nc = bacc.Bacc("TRN2", target_bir_lowering=False, debug=False)
        build(nc, debug=debug, repeat=repeat, dyn_loop=dyn_loop, with_cbias=with_cbias)
        nc.compile()
        _CACHE[key] = nc
    return _CACHE[key]


def prep_inputs(x, qkv_w, proj_w, proj_b, conv_ws, conv_bs):
    x = np.asarray(x)
    B = x.shape[0]
    xb = x.astype(ml_dtypes.bfloat16)
    qwb = np.asarray(qkv_w).astype(ml_dtypes.bfloat16)
    pwb = np.asarray(proj_w).astype(ml_dtypes.bfloat16)
    w9 = np.asarray(conv_ws).reshape(4, 128, 9).astype(np.float32)
    dgn = np.zeros((36, 128, 128), np.float32)
    idx = np.arange(128)
    for br in range(4):
        for t, (dr, dj) in enumerate(TAPS):
            dgn[br * 9 + t, idx, idx] = w9[br, :, (dr + 1) * 3 + (dj + 1)]
    dgn = dgn.astype(ml_dtypes.bfloat16)
    pbb = np.tile(np.asarray(proj_b, np.float32)[None, :], (128, 1))
    cbt = np.ascontiguousarray(np.asarray(conv_bs, np.float32).T)
    shared = {"qw": qwb, "pw": pwb, "dg": dgn, "pb": pbb, "cb": cbt}
    return [dict(shared, xb=np.ascontiguousarray(xb[b])) for b in range(B)]


def kernel(x, qkv_w, proj_w, proj_b, conv_ws, conv_bs, _debug=False, _trace=False):
    wcb = bool(np.any(np.asarray(conv_bs)))
    nc = _get_nc(debug=_debug, with_cbias=wcb)
    in_maps = prep_inputs(x, qkv_w, proj_w, proj_b, conv_ws, conv_bs)
    res = run_bass_kernel_spmd(nc, in_maps, core_ids=list(range(len(in_maps))),
                               trace=_trace)
    out = np.stack([r["out"] for r in res.results]).astype(np.float32)
    if _debug or _trace:
        kernel.last_results = res
    return out
